# revision 15
# baseline (speedup 1.0000x reference)
"""Trainium2 Bass kernel for the DiffRenderer problem.

Math (per grid cell): probs = softmax(grid_logits[r, c, :]); each cell's
28x14 tile = sum_n probs[n] * font[n]; tiles assembled into a (10752, 10752)
image.

Strategy (8 cores, data-parallel over grid rows — 48 rows per core):
  - Host shards grid_logits by row band and lays each band out as
    logitsT [69 chars, 36864 cells] so the char axis is on SBUF partitions.
  - Per core, per load tile of 3072 cells:
      * DMA load logitsT [69, 3072] (ACT HWDGE ring)
      * ACT exp -> float32r SBUF tile (softmax without max-subtract:
        logits are N(0,1), exp is safely within fp32 range)
      * per 128-cell chunk: PE matmul (float32r, single-pass full rate)
        expT.T @ [font | ones | 0] -> PSUM [128 cells, 394]; column 392 is
        the softmax denominator (ones column), 393 keeps counts even as
        required by fp32r matmuls
      * DVE reciprocal of the denominator; normalization fused into the
        PSUM->SBUF copy (DVE tensor_scalar / ACT copy-with-scale, alternating)
      * DMA out one 4.8MB fully-contiguous block per load tile (SP ring)
  - Device output is the reference's soft_tiles data in a DMA-friendly
    permutation; the host performs the pure reindex to image form — the
    same transpose/reshape the reference itself performs after the math.
"""

import os
from contextlib import ExitStack

import numpy as np

os.environ.setdefault("MYCRO_LOCAL_CACHE", "1")

import concourse.bass as bass  # noqa: F401
import concourse.tile as tile
from concourse import bacc, mybir
from concourse.bass_utils import run_bass_kernel_spmd


def _install_ntff_hook_shim():
    """The image's antenv lacks axon_hooks, but run_bass_kernel_spmd imports
    it whenever BASS_TRACE is set. Provide the module and register the
    ctypes-based NTFF profile hook (degrades to no tracing if unavailable)."""
    import sys
    import types

    if "antenv.axon_hooks" in sys.modules:
        return
    try:
        import antenv
    except ImportError:
        return
    mod = types.ModuleType("antenv.axon_hooks")
    mod._hook = None
    mod.set_axon_ntff_profile_hook = lambda h: setattr(mod, "_hook", h)
    mod.get_axon_ntff_profile_hook = lambda: mod._hook
    sys.modules["antenv.axon_hooks"] = mod
    antenv.axon_hooks = mod
    try:
        from trn_agent_boot.trn_boot import _ntff_profile_via_ctypes

        hook = _ntff_profile_via_ctypes("/opt/axon/libaxon_pjrt.so")
        if hook is not None:
            mod.set_axon_ntff_profile_hook(hook)
    except Exception:
        pass


_install_ntff_hook_shim()

# Problem constants (hardcoded per harness contract)
ROWS, COLS, N_CHARS = 384, 768, 69
CH, CW = 28, 14
HW = CH * CW  # 392
NPAD = HW + 2  # col 392 = ones (softmax denom); 393 = pad (fp32r wants even N)
N_CORES = 8
ROWS_PER_CORE = ROWS // N_CORES  # 48
CELLS = ROWS_PER_CORE * COLS  # 36864 cells per core
P = 128  # matmul output partitions (cells per chunk)
J = 24  # chunks per load tile
CT = P * J  # 3072 cells per load tile
T = CELLS // CT  # 12 load tiles per core
ES = 4  # exp sub-chunks per load tile
F32 = mybir.dt.float32
F32R = mybir.dt.float32r
F16 = mybir.dt.float16

# Stash of the last run's BassKernelResults (test.py reads exec_time_ns).
LAST_RESULTS = None
_CACHED_NC = None


def _build_bass():
    nc = bacc.Bacc("TRN2", target_bir_lowering=False, debug=False,
                   num_devices=N_CORES)

    logits_h = nc.dram_tensor("logitsT", [T, N_CHARS, CT], F32,
                              kind="ExternalInput")
    fontb_h = nc.dram_tensor("fontb", [N_CHARS, NPAD], F16,
                             kind="ExternalInput")
    # out[t, p, j, :] holds soft_tiles for cell t*3072 + j*128 + p.
    # fp16: halves the dominant HBM-write traffic; adds ~2.4e-4 rounding
    out_h = nc.dram_tensor("out", [CELLS, HW], F16, kind="ExternalOutput")

    with tile.TileContext(nc) as tc, ExitStack() as ctx:
        singles = ctx.enter_context(tc.tile_pool(name="singles", bufs=1))
        inp = ctx.enter_context(tc.tile_pool(name="inp", bufs=4))
        expp = ctx.enter_context(tc.tile_pool(name="expp", bufs=3))
        outp = ctx.enter_context(tc.tile_pool(name="outp", bufs=4))
        rcpp = ctx.enter_context(tc.tile_pool(name="rcpp", bufs=8))
        # [128, 1024] = two PSUM banks per tile; two matmuls (cols 0 and 512)
        # share one tile so one reciprocal handles both denominators
        ps_m = ctx.enter_context(tc.tile_pool(name="ps_m", bufs=4,
                                              space="PSUM"))

        fontb_sb = singles.tile([N_CHARS, NPAD], F16)
        nc.scalar.dma_start(fontb_sb, fontb_h[:])

        logits_v = logits_h[:]
        out_v = out_h[:].rearrange("(t p j) f -> t p (j f)", p=P, j=J)

        for t in range(T):
            in_tile = inp.tile([N_CHARS, CT], F32)
            # SWDGE: 69-partition transfers collapse to 3 engines on the
            # HWDGE rings; gpsimd's ring spreads well. Split loads so the
            # first exp (and the pipeline) starts sooner.
            nsub = 4 if t == 0 else 2
            for sb in range(nsub):
                w = CT // nsub
                nc.gpsimd.dma_start(in_tile[:, sb * w:(sb + 1) * w],
                                    logits_v[t][:, sb * w:(sb + 1) * w])
            eT = expp.tile([N_CHARS, CT], F16)
            for s in range(ES):
                sl = slice(s * (CT // ES), (s + 1) * (CT // ES))
                nc.scalar.activation(eT[:, sl], in_tile[:, sl],
                                     mybir.ActivationFunctionType.Exp)
            out_tile = outp.tile([P, J * HW], F16)
            for jp in range(J // 2):
                j0, j1 = 2 * jp, 2 * jp + 1
                psm = ps_m.tile([P, 1024], F32)
                nc.tensor.matmul(psm[:, 0:NPAD],
                                 eT[:, j0 * P:(j0 + 1) * P], fontb_sb[:],
                                 start=True, stop=True)
                nc.tensor.matmul(psm[:, 512:512 + NPAD],
                                 eT[:, j1 * P:(j1 + 1) * P], fontb_sb[:],
                                 start=True, stop=True)
                rc = rcpp.tile([P, 2], F32)
                psm_v = psm[:].rearrange("p (two f) -> p two f", two=2)
                nc.vector.reciprocal(
                    rc[:].rearrange("p (two f) -> p two f", two=2),
                    psm_v[:, :, HW:HW + 1])
                for j, base in ((j0, 0), (j1, 512)):
                    dst = out_tile[:, j * HW:(j + 1) * HW]
                    sc = rc[:, j - j0:j - j0 + 1]
                    if j % 5 < 3:
                        nc.vector.tensor_scalar_mul(
                            dst, psm[:, base:base + HW], sc)
                    else:
                        nc.scalar.mul(dst, psm[:, base:base + HW], sc)
                if jp == J // 4 - 1:
                    nc.sync.dma_start(out_v[t][:, :J * HW // 2],
                                      out_tile[:, :J * HW // 2])
            nc.sync.dma_start(out_v[t][:, J * HW // 2:],
                              out_tile[:, J * HW // 2:])

    nc.compile()
    return nc


def kernel(grid_logits: np.ndarray, font: np.ndarray) -> np.ndarray:
    global LAST_RESULTS, _CACHED_NC
    grid_logits = np.asarray(grid_logits, dtype=np.float32)
    font = np.asarray(font, dtype=np.float32)
    assert grid_logits.shape == (ROWS, COLS, N_CHARS)
    assert font.shape == (N_CHARS, CH, CW)

    fontb = np.zeros((N_CHARS, NPAD), dtype=np.float32)
    fontb[:, :HW] = font.reshape(N_CHARS, HW)
    fontb[:, HW] = 1.0
    fontb = fontb.astype(np.float16)

    # (69, 384, 768) with chars leading: one big transpose, then per-core
    # contiguous band slices
    glT = np.ascontiguousarray(grid_logits.transpose(2, 0, 1))

    in_maps = []
    for k in range(N_CORES):
        band = glT[:, k * ROWS_PER_CORE:(k + 1) * ROWS_PER_CORE, :]
        bandc = np.ascontiguousarray(band).reshape(N_CHARS, T, CT)
        in_maps.append({
            "logitsT": np.ascontiguousarray(bandc.transpose(1, 0, 2)),
            "fontb": fontb,
        })

    if _CACHED_NC is None:
        _CACHED_NC = _build_bass()

    res = run_bass_kernel_spmd(_CACHED_NC, in_maps,
                               core_ids=list(range(N_CORES)))
    LAST_RESULTS = res

    img = np.empty((ROWS * CH, COLS * CW), dtype=np.float32)
    band_h = ROWS_PER_CORE * CH  # 1344
    for k in range(N_CORES):
        arr = res.results[k]["out"].reshape(T, P, J, CH, CW)
        # [t, p, j] holds cell t*3072 + j*128 + p -> reorder to cell-major
        cells = arr.transpose(0, 2, 1, 3, 4).reshape(
            ROWS_PER_CORE, COLS, CH, CW)
        img[k * band_h:(k + 1) * band_h] = (
            cells.transpose(0, 2, 1, 3).reshape(band_h, COLS * CW))
    return img[None, None]
